# revision 1
# baseline (speedup 1.0000x reference)
"""Agent-Attention kernel for 8 Trainium2 NeuronCores.

Strategy: data-parallel over batch (b=32 -> 4 per core), per the sharding
hint.  The full computation (qkv proj, agent pooling, two-stage agent
attention, depthwise-conv residual, output proj) is compiled once with
jax.pmap across the 8 axon-tunneled NeuronCores; inputs are sharded over
batch, weights broadcast, and the output gathered to full shape.

Falls back to a pure-numpy host implementation if the device path is
unavailable, so kernel(**inputs) always returns the correct full output.
"""

import numpy as np

# Problem shapes (hardcoded per spec; kernel.py must be self-contained).
B, N, C = 32, 3136, 512
H = W = 56
NUM_HEADS = 8
HEAD_DIM = C // NUM_HEADS
AGENT_NUM = 49
POOL = 7
SCALE = HEAD_DIM ** -0.5
N_CORES = 8


def _agent_attention_jax(x, qkv_w, proj_w, proj_b, dwc_w, dwc_b):
    """Per-shard computation. x: (b_local, N, C)."""
    import jax
    import jax.numpy as jnp

    b, n, c = x.shape
    qkv = jnp.einsum('bnc,dc->bnd', x, qkv_w).reshape(b, n, 3, c)
    q, k, v = qkv[:, :, 0], qkv[:, :, 1], qkv[:, :, 2]

    s = H // POOL
    agent = q.reshape(b, POOL, s, POOL, s, c).mean(axis=(2, 4)).reshape(b, AGENT_NUM, c)

    qh = q.reshape(b, n, NUM_HEADS, HEAD_DIM).transpose(0, 2, 1, 3)
    kh = k.reshape(b, n, NUM_HEADS, HEAD_DIM).transpose(0, 2, 1, 3)
    vh = v.reshape(b, n, NUM_HEADS, HEAD_DIM).transpose(0, 2, 1, 3)
    ah = agent.reshape(b, AGENT_NUM, NUM_HEADS, HEAD_DIM).transpose(0, 2, 1, 3)

    agent_attn = jax.nn.softmax(jnp.einsum('bhad,bhnd->bhan', ah * SCALE, kh), axis=-1)
    agent_v = jnp.einsum('bhan,bhnd->bhad', agent_attn, vh)

    q_attn = jax.nn.softmax(jnp.einsum('bhnd,bhad->bhna', qh * SCALE, ah), axis=-1)
    out = jnp.einsum('bhna,bhad->bhnd', q_attn, agent_v)
    out = out.transpose(0, 2, 1, 3).reshape(b, n, c)

    v_img = v.reshape(b, H, W, c).transpose(0, 3, 1, 2)
    dwc = jax.lax.conv_general_dilated(
        v_img, dwc_w, window_strides=(1, 1), padding=((1, 1), (1, 1)),
        dimension_numbers=('NCHW', 'OIHW', 'NCHW'), feature_group_count=c)
    dwc = dwc + dwc_b[None, :, None, None]
    out = out + dwc.transpose(0, 2, 3, 1).reshape(b, n, c)

    return jnp.einsum('bnc,dc->bnd', out, proj_w) + proj_b


def _run_on_neuron(inputs):
    import jax

    devs = [d for d in jax.devices() if d.platform != 'cpu'][:N_CORES]
    if len(devs) < N_CORES:
        raise RuntimeError(f'need {N_CORES} accelerator devices, found {len(devs)}')

    fn = jax.pmap(
        _agent_attention_jax,
        in_axes=(0, None, None, None, None, None),
        devices=devs,
    )
    xs = np.ascontiguousarray(
        inputs['x'].reshape(N_CORES, B // N_CORES, N, C))
    out = fn(xs, inputs['qkv_w'], inputs['proj_w'], inputs['proj_b'],
             inputs['dwc_w'], inputs['dwc_b'])
    return np.asarray(out).reshape(B, N, C)


def _run_numpy(inputs):
    x = np.asarray(inputs['x'], dtype=np.float32)
    qkv_w = np.asarray(inputs['qkv_w'], dtype=np.float32)
    proj_w = np.asarray(inputs['proj_w'], dtype=np.float32)
    proj_b = np.asarray(inputs['proj_b'], dtype=np.float32)
    dwc_w = np.asarray(inputs['dwc_w'], dtype=np.float32)
    dwc_b = np.asarray(inputs['dwc_b'], dtype=np.float32)

    b, n, c = x.shape
    qkv = (x.reshape(b * n, c) @ qkv_w.T).reshape(b, n, 3, c)
    q = np.ascontiguousarray(qkv[:, :, 0])
    k = np.ascontiguousarray(qkv[:, :, 1])
    v = np.ascontiguousarray(qkv[:, :, 2])

    s = H // POOL
    agent = q.reshape(b, POOL, s, POOL, s, c).mean(axis=(2, 4)).reshape(b, AGENT_NUM, c)

    qh = q.reshape(b, n, NUM_HEADS, HEAD_DIM).transpose(0, 2, 1, 3)
    kh = k.reshape(b, n, NUM_HEADS, HEAD_DIM).transpose(0, 2, 1, 3)
    vh = v.reshape(b, n, NUM_HEADS, HEAD_DIM).transpose(0, 2, 1, 3)
    ah = agent.reshape(b, AGENT_NUM, NUM_HEADS, HEAD_DIM).transpose(0, 2, 1, 3)

    def softmax(z):
        z = z - z.max(axis=-1, keepdims=True)
        ez = np.exp(z)
        return ez / ez.sum(axis=-1, keepdims=True)

    agent_attn = softmax(np.einsum('bhad,bhnd->bhan', ah * SCALE, kh))
    agent_v = np.einsum('bhan,bhnd->bhad', agent_attn, vh)

    q_attn = softmax(np.einsum('bhnd,bhad->bhna', qh * SCALE, ah))
    out = np.einsum('bhna,bhad->bhnd', q_attn, agent_v)
    out = out.transpose(0, 2, 1, 3).reshape(b, n, c)

    # depthwise 3x3, padding 1, on v laid out as (b, H, W, c)
    v_img = v.reshape(b, H, W, c)
    vp = np.zeros((b, H + 2, W + 2, c), dtype=np.float32)
    vp[:, 1:-1, 1:-1, :] = v_img
    dwc = np.zeros((b, H, W, c), dtype=np.float32)
    for di in range(3):
        for dj in range(3):
            dwc += vp[:, di:di + H, dj:dj + W, :] * dwc_w[:, 0, di, dj]
    dwc += dwc_b
    out = out + dwc.reshape(b, n, c)

    return (out.reshape(b * n, c) @ proj_w.T).reshape(b, n, c) + proj_b


def kernel(**inputs) -> np.ndarray:
    try:
        out = _run_on_neuron(inputs)
    except Exception:
        out = _run_numpy(inputs)
    return np.asarray(out, dtype=np.float32)


if __name__ == '__main__':
    rng = np.random.default_rng(0)
    demo = {
        'x': rng.standard_normal((B, N, C), dtype=np.float32),
        'qkv_w': rng.standard_normal((3 * C, C), dtype=np.float32) * 0.02,
        'proj_w': rng.standard_normal((C, C), dtype=np.float32) * 0.02,
        'proj_b': rng.standard_normal(C).astype(np.float32) * 0.02,
        'dwc_w': rng.standard_normal((C, 1, 3, 3)).astype(np.float32) * 0.02,
        'dwc_b': rng.standard_normal(C).astype(np.float32) * 0.02,
    }
    print(kernel(**demo).shape)
